# revision 11
# baseline (speedup 1.0000x reference)
"""Trainium2 Bass kernel: per-batch segment-mean pooling + 3-layer MLP.

Reference computation (B=64, T=512, H=768, S=128):
  pooled[b,s,:] = mean over t of hidden[b,t,:] where statements_ids[b,t]==s
  x = gelu(pooled @ w1 + b1); x = gelu(x @ w2 + b2)
  out[b,s] = sigmoid(x @ w3 + b3)

Distribution: data-parallel over batch across 8 NeuronCores (8 batches per
core); MLP weights replicated.

All large operands are cast to fp16 on the host (tolerance is 2e-2 rel;
fp16 end-to-end measures ~3.5e-4), halving HBM traffic to ~8.4 MB/core and
letting every PE matmul run at 1 cycle/row at any moving width.

Per-core algorithm (transposed pooling — no PE transposes, no separate
normalization):
  - The scaled one-hot M[t,s] = (sid[t]==s) * (1/count[sid[t]]) is built in
    one DVE tensor_scalar op per t-tile: (iota == sid) * invtok, where sid
    and invtok are per-partition f32 scalar operands packed on the host
    (segment counts are metadata of the int32 index input).
  - pooled^T[m-block] = hidden[b]^T @ M via PE with the hidden k/m-block as
    the stationary operand and M as the moving operand: the psum result is
    the normalized, transposed MLP input directly. m-blocks rotate over 3
    psum banks (m%3) so the per-m PSUM->SBUF copy (alternating DVE/ACT)
    overlaps the next m-blocks' matmuls on other banks, and sequential
    accumulation groups per bank keep the has_written semantics safe.
  - MLP batched over all 8 local batches: rows = 8*128 = 1024 moving dim,
    weights stationary; gelu/sigmoid + bias fused on ACT. fc1 runs in
    256-wide chunks interleaved with pooling (DMA overlap); fc2/fc3 in
    512-wide chunks interleaved as fc2c0, fc3c0, fc2c1, fc3c1 so the first
    output DMA overlaps the remaining compute.

Schedule notes (from NTFF profiling):
  - The framework preamble ends ~7us in; ~30 garbage-input warm-up
    transposes keep the PE busy until the first hidden batch lands, pulling
    the HAM clock gate to 8/8 (2.4 GHz) before real matmuls start.
  - All DMAs issue on the sync engine (~0.7us each): consts, h0, h1, w1(x2),
    h2, h3, h4, w2a, h5, w2b, h6, h7 — ordered so each arrives just before
    its consumer needs it.
"""

import os
import sys

sys.path.insert(0, "/opt/trn_rl_repo")

import numpy as np

import concourse.bass as bass
import concourse.mybir as mybir
import concourse.tile as tile
from concourse import bacc, bass_utils

B, T, H, S = 64, 512, 768, 128
N_CORES = 8
BL = B // N_CORES  # local batches per core
P = 128
KT = T // P        # t-tiles per batch
KH = H // P        # h-tiles
R = BL * S         # MLP rows per core
RC1 = 2 * S        # fc1 moving-dim chunk (2 batches)
RC2 = 4 * S        # fc2/fc3 moving-dim chunk (4 batches, full psum bank)
N_WARMUP = 34      # PE warm-up matmuls during the DMA ramp

# merged constant pack, one DMA, f32 element view [P, CC_COLS]:
#   [0:64)    iota (128 f16 cols)    [64:67)  w3 (6 f16 cols)
#   [67:73)   b1   [73:79) b2   [79:80) b3    (f32)
#   [80:112)  sid    (f32, col b*KT+k = sid of token k*128+p)
#   [112:144) invtok (f32, col b*KT+k = 1/count[sid] of that token)
CC_COLS = 144

_CACHE: dict = {}


def _build_program():
    f32, f16 = mybir.dt.float32, mybir.dt.float16
    FT = mybir.ActivationFunctionType
    OP = mybir.AluOpType

    nc = bacc.Bacc("TRN2", target_bir_lowering=False, debug=False)
    hid = nc.dram_tensor("hidden", [BL, T, H], f16, kind="ExternalInput").ap()
    w1 = nc.dram_tensor("w1", [H, H], f16, kind="ExternalInput").ap()
    w2 = nc.dram_tensor("w2", [H, H], f16, kind="ExternalInput").ap()
    cpack = nc.dram_tensor("cpack", [P, CC_COLS], f32, kind="ExternalInput").ap()
    out = nc.dram_tensor("out", [BL, S], f32, kind="ExternalOutput").ap()
    dbg = nc.dram_tensor("dbg", [1, 1], f32, kind="ExternalOutput").ap()

    with tile.TileContext(nc) as tc:
        with (
            tc.tile_pool(name="consts", bufs=1) as consts,
            tc.tile_pool(name="wpool", bufs=1) as wpool,
            tc.tile_pool(name="hpool", bufs=1) as hpool,
            tc.tile_pool(name="mtpool", bufs=8) as mtpool,
            tc.tile_pool(name="small", bufs=2) as small,
            tc.tile_pool(name="xtpool", bufs=1) as xtpool,
            tc.tile_pool(name="ypool", bufs=1) as ypool,
            tc.tile_pool(name="psA", bufs=6, space="PSUM") as psA,
            tc.tile_pool(name="psF", bufs=2, space="PSUM") as psF,
        ):
            # ---- PE warm-up: transposes on a memset scratch tile, no DMA
            # deps, so they run the moment the preamble ends and pull the
            # HAM clock gate to 8/8 before real matmuls arrive. A [1,1]
            # copy of the last result feeds the dbg output (keeps DCE off);
            # its DMA is issued at the very end of the program. ----
            wu_sb = small.tile([P, P], f16, tag="wu_src")
            nc.gpsimd.memset(wu_sb, 0.0)
            wu_ps = None

            def warmup(n):
                nonlocal wu_ps
                for i in range(n):
                    wu_ps = psF.tile([P, P], f32, tag="ps")
                    nc.tensor.matmul(wu_ps, lhsT=wu_sb, rhs=wu_sb, start=True, stop=True)

            warmup(N_WARMUP)
            dbg_sb = small.tile([1, 1], f32, tag="dbg")

            cc_sb = consts.tile([P, CC_COLS], f32)
            nc.sync.dma_start(cc_sb, cpack)
            iota_sb = cc_sb[:, 0:64].bitcast(f16)
            w3_sb = cc_sb[:, 64:67].bitcast(f16)
            b1_sb = cc_sb[:, 67:73]
            b2_sb = cc_sb[:, 73:79]
            b3_sb = cc_sb[0:1, 79:80]
            sid_sb = cc_sb[:, 80:112]
            ivt_sb = cc_sb[:, 112:144]

            # ---- hidden + weight streaming on sync/HWDGE, ordered to match
            # the compute pipeline ----
            hbs = [None] * BL
            w1ks = [None] * KH
            w2ks = [None] * KH

            def load_hb(b):
                hb = hpool.tile(
                    [P, KT, H], f16, tag=f"hb{b % 5}", name=f"hb{b}"
                )
                nc.sync.dma_start(hb, hid[b].rearrange("(p k) h -> p k h", p=P))
                hbs[b] = hb

            def load_w(ws, wdram, k0, nm):
                # one DMA for three k-tiles
                t = wpool.tile([P, 3, H], f16, tag=f"{nm}{k0}", name=f"{nm}{k0}")
                nc.sync.dma_start(
                    t, wdram[k0 * P : (k0 + 3) * P, :].rearrange("(k p) h -> p k h", p=P)
                )
                for k in range(3):
                    ws[k0 + k] = t[:, k, :]

            load_hb(0)
            load_hb(1)
            load_w(w1ks, w1, 0, "w1k")
            load_w(w1ks, w1, 3, "w1k")
            load_hb(2)
            load_hb(3)
            load_hb(4)
            load_w(w2ks, w2, 0, "w2k")
            load_hb(5)
            load_w(w2ks, w2, 3, "w2k")
            load_hb(6)
            load_hb(7)

            xts = [xtpool.tile([P, R], f16, tag=f"xt{k}", name=f"xt{k}") for k in range(KH)]
            y1s = [ypool.tile([P, R], f16, tag=f"y1_{m}", name=f"y1_{m}") for m in range(KH)]
            y2s = [ypool.tile([P, R], f16, tag=f"y2_{m}", name=f"y2_{m}") for m in range(KH)]
            pred = ypool.tile([1, R], f32, tag="pred")

            def pool(b):
                # scaled one-hots for this batch's four t-tiles
                mts = []
                for k in range(KT):
                    mt = mtpool.tile([P, P], f16, tag="mt")
                    nc.vector.tensor_scalar(
                        mt, iota_sb,
                        sid_sb[:, b * KT + k : b * KT + k + 1],
                        ivt_sb[:, b * KT + k : b * KT + k + 1],
                        OP.is_equal, OP.mult,
                    )
                    mts.append(mt)
                # pooled^T m-blocks: psum banks rotate m%3 so the per-m copy
                # (on the other engines) never touches the bank PE is writing
                pts = [
                    psA.tile([P, 2 * P], f32, tag="psA", name=f"pool{b}_{j}")
                    for j in range(3)
                ]
                for m in range(KH):
                    dst = pts[m % 3][:, (m // 3) * P : (m // 3 + 1) * P]
                    for k in range(KT):
                        nc.tensor.matmul(
                            dst,
                            lhsT=hbs[b][:, k, m * P : (m + 1) * P],
                            rhs=mts[k],
                            start=(k == 0),
                            stop=(k == KT - 1),
                        )
                    # copy this m-block out while the next m-blocks stream
                    if m % 2 == 0:
                        nc.vector.tensor_copy(xts[m][:, b * S : (b + 1) * S], dst)
                    else:
                        nc.scalar.copy(xts[m][:, b * S : (b + 1) * S], dst)

            def fc(wks, b_sb, xs, outs, rc, RC, func):
                for m in range(KH):
                    pt = psF.tile([P, RC], f32, tag="ps")
                    for k in range(KH):
                        nc.tensor.matmul(
                            pt,
                            lhsT=wks[k][:, m * P : (m + 1) * P],
                            rhs=xs[k][:, rc * RC : (rc + 1) * RC],
                            start=(k == 0),
                            stop=(k == KH - 1),
                        )
                    nc.scalar.activation(
                        outs[m][:, rc * RC : (rc + 1) * RC],
                        pt,
                        func,
                        bias=b_sb[:, m : m + 1],
                    )

            def fc3(rc):
                pt = psF.tile([1, RC2], f32, tag="ps")
                for k in range(KH):
                    nc.tensor.matmul(
                        pt,
                        lhsT=w3_sb[:, k : k + 1],
                        rhs=y2s[k][:, rc * RC2 : (rc + 1) * RC2],
                        start=(k == 0),
                        stop=(k == KH - 1),
                    )
                nc.scalar.activation(
                    pred[:, rc * RC2 : (rc + 1) * RC2],
                    pt,
                    FT.Sigmoid,
                    bias=b3_sb,
                )
                nc.sync.dma_start(
                    out.rearrange("b s -> (b s)")[rc * RC2 : (rc + 1) * RC2],
                    pred[:, rc * RC2 : (rc + 1) * RC2],
                )

            gelu = FT.Gelu
            pool(0)
            # keep the PE busy across the h1 arrival gap so the HAM activity
            # window sees one uninterrupted span and unthrottles early
            warmup(10)
            nc.vector.tensor_copy(dbg_sb, wu_ps[0:1, 0:1])
            pool(1)
            fc(w1ks, b1_sb, xts, y1s, 0, RC1, gelu)
            pool(2)
            pool(3)
            fc(w1ks, b1_sb, xts, y1s, 1, RC1, gelu)
            pool(4)
            pool(5)
            fc(w1ks, b1_sb, xts, y1s, 2, RC1, gelu)
            pool(6)
            fc(w2ks, b2_sb, y1s, y2s, 0, RC2, gelu)
            pool(7)
            fc(w1ks, b1_sb, xts, y1s, 3, RC1, gelu)
            fc(w2ks, b2_sb, y1s, y2s, 1, RC2, gelu)
            fc3(0)
            fc3(1)

            nc.sync.dma_start(dbg, dbg_sb)

    nc.compile()
    return nc


def _get_program():
    if "nc" not in _CACHE:
        _CACHE["nc"] = _build_program()
    return _CACHE["nc"]


def _cpack(sid_shard, b1, b2, b3, w3):
    """Merged per-core constant pack (one DMA): fp16 matmul operands and
    f32 biases/sid/invtok, byte-concatenated per partition row."""
    h16 = np.zeros((P, P + KH), dtype=np.float16)
    h16[:, 0:P] = np.arange(P, dtype=np.float16)[None, :]
    h16[:, P:] = (
        np.asarray(w3, np.float32).reshape(KH, P, 1)[:, :, 0].T.astype(np.float16)
    )
    f = np.zeros((P, 77), dtype=np.float32)
    f[:, 0:6] = np.asarray(b1, np.float32).reshape(KH, P).T
    f[:, 6:12] = np.asarray(b2, np.float32).reshape(KH, P).T
    f[0, 12] = np.float32(np.asarray(b3).reshape(-1)[0])
    # token layout matches the hidden DMA: partition p, col k = token 4p+k
    for b in range(BL):
        sid_pk = sid_shard[b].astype(np.int64).reshape(P, KT)
        f[:, 13 + b * KT : 13 + (b + 1) * KT] = sid_pk.astype(np.float32)
        # per-token inverse segment size (counts are metadata of the int32
        # index input): invtok[t] = 1/count[sid[t]]
        cnt = np.bincount(sid_shard[b].astype(np.int64), minlength=S)[:S]
        invb = 1.0 / np.maximum(cnt, 1).astype(np.float32)
        f[:, 45 + b * KT : 45 + (b + 1) * KT] = invb[sid_pk]
    row_bytes = np.concatenate(
        [h16.view(np.uint8).reshape(P, -1), f.view(np.uint8).reshape(P, -1)], axis=1
    )
    return np.ascontiguousarray(row_bytes).view(np.float32)


def make_in_maps(hidden, statements_ids, w1, b1, w2, b2, w3, b3):
    hidden = np.ascontiguousarray(
        np.asarray(hidden, dtype=np.float32).astype(np.float16)
    )
    sid = np.asarray(statements_ids, dtype=np.int32)
    w1 = np.ascontiguousarray(np.asarray(w1, dtype=np.float32).astype(np.float16))
    w2 = np.ascontiguousarray(np.asarray(w2, dtype=np.float32).astype(np.float16))
    in_maps = []
    for c in range(N_CORES):
        cc = _cpack(sid[c * BL : (c + 1) * BL], b1, b2, b3, w3)
        in_maps.append(
            {
                "hidden": hidden[c * BL : (c + 1) * BL],
                "w1": w1,
                "w2": w2,
                "cpack": cc,
            }
        )
    return in_maps


def kernel(hidden, statements_ids, w1, b1, w2, b2, w3, b3, **kwargs):
    nc = _get_program()
    in_maps = make_in_maps(hidden, statements_ids, w1, b1, w2, b2, w3, b3)
    trace = bool(int(os.environ.get("KERNEL_TRACE", "0")))
    res = bass_utils.run_bass_kernel_spmd(
        nc, in_maps, core_ids=list(range(N_CORES)), trace=trace
    )
    _CACHE["last_results"] = res
    out = np.concatenate([res.results[c]["out"] for c in range(N_CORES)], axis=0)
    return out.astype(np.float32)


# revision 14
# speedup vs baseline: 1.0016x; 1.0016x over previous
"""Trainium2 Bass kernel: per-batch segment-mean pooling + 3-layer MLP.

Reference computation (B=64, T=512, H=768, S=128):
  pooled[b,s,:] = mean over t of hidden[b,t,:] where statements_ids[b,t]==s
  x = gelu(pooled @ w1 + b1); x = gelu(x @ w2 + b2)
  out[b,s] = sigmoid(x @ w3 + b3)

Distribution: data-parallel over batch across 8 NeuronCores (8 batches per
core); MLP weights replicated.

All large operands are cast to fp16 on the host (tolerance is 2e-2 rel;
fp16 end-to-end measures ~3.5e-4), halving HBM traffic to ~8.4 MB/core and
letting every PE matmul run at 1 cycle/row at any moving width.

Per-core algorithm (transposed pooling — no PE transposes, no separate
normalization):
  - The scaled one-hot M[t,s] = (sid[t]==s) * (1/count[sid[t]]) is built in
    one DVE tensor_scalar op per t-tile: (iota == sid) * invtok, where sid
    and invtok are per-partition f32 scalar operands packed on the host
    (segment counts are metadata of the int32 index input).
  - pooled^T[m-block] = hidden[b]^T @ M via PE with the hidden k/m-block as
    the stationary operand and M as the moving operand: the psum result is
    the normalized, transposed MLP input directly. m-blocks rotate over 3
    psum banks (m%3) so the per-m PSUM->SBUF copy (alternating DVE/ACT)
    overlaps the next m-blocks' matmuls on other banks, and sequential
    accumulation groups per bank keep the has_written semantics safe.
  - MLP batched over all 8 local batches: rows = 8*128 = 1024 moving dim,
    weights stationary; gelu/sigmoid + bias fused on ACT. fc1 runs in
    256-wide chunks interleaved with pooling (DMA overlap); fc2/fc3 in
    512-wide chunks interleaved as fc2c0, fc3c0, fc2c1, fc3c1 so the first
    output DMA overlaps the remaining compute.

Schedule notes (from NTFF profiling):
  - The framework preamble ends ~7us in; ~30 garbage-input warm-up
    transposes keep the PE busy until the first hidden batch lands, pulling
    the HAM clock gate to 8/8 (2.4 GHz) before real matmuls start.
  - All DMAs issue on the sync engine (~0.7us each): consts, h0, h1, w1(x2),
    h2, h3, h4, w2a, h5, w2b, h6, h7 — ordered so each arrives just before
    its consumer needs it.
"""

import os
import sys

sys.path.insert(0, "/opt/trn_rl_repo")

import numpy as np

import concourse.bass as bass
import concourse.mybir as mybir
import concourse.tile as tile
from concourse import bacc, bass_utils

B, T, H, S = 64, 512, 768, 128
N_CORES = 8
BL = B // N_CORES  # local batches per core
P = 128
KT = T // P        # t-tiles per batch
KH = H // P        # h-tiles
R = BL * S         # MLP rows per core
RC1 = 2 * S        # fc1 moving-dim chunk (2 batches)
RC2 = 4 * S        # fc2/fc3 moving-dim chunk (4 batches, full psum bank)
N_WARMUP = 30      # PE warm-up matmuls during the DMA ramp

# merged constant pack, one DMA, f32 element view [P, CC_COLS]:
#   [0:64)    iota (128 f16 cols)    [64:67)  w3 (6 f16 cols)
#   [67:73)   b1   [73:79) b2   [79:80) b3    (f32)
#   [80:112)  sid    (f32, col b*KT+k = sid of token k*128+p)
#   [112:144) invtok (f32, col b*KT+k = 1/count[sid] of that token)
CC_COLS = 144

_CACHE: dict = {}


def _build_program():
    f32, f16 = mybir.dt.float32, mybir.dt.float16
    FT = mybir.ActivationFunctionType
    OP = mybir.AluOpType

    nc = bacc.Bacc("TRN2", target_bir_lowering=False, debug=False)
    hid = nc.dram_tensor("hidden", [BL, T, H], f16, kind="ExternalInput").ap()
    w1 = nc.dram_tensor("w1", [H, H], f16, kind="ExternalInput").ap()
    w2 = nc.dram_tensor("w2", [H, H], f16, kind="ExternalInput").ap()
    cpack = nc.dram_tensor("cpack", [P, CC_COLS], f32, kind="ExternalInput").ap()
    out = nc.dram_tensor("out", [BL, S], f32, kind="ExternalOutput").ap()
    dbg = nc.dram_tensor("dbg", [1, 1], f32, kind="ExternalOutput").ap()

    with tile.TileContext(nc) as tc:
        with (
            tc.tile_pool(name="consts", bufs=1) as consts,
            tc.tile_pool(name="wpool", bufs=1) as wpool,
            tc.tile_pool(name="hpool", bufs=1) as hpool,
            tc.tile_pool(name="mtpool", bufs=8) as mtpool,
            tc.tile_pool(name="small", bufs=2) as small,
            tc.tile_pool(name="xtpool", bufs=1) as xtpool,
            tc.tile_pool(name="ypool", bufs=1) as ypool,
            tc.tile_pool(name="psA", bufs=6, space="PSUM") as psA,
            tc.tile_pool(name="psF", bufs=2, space="PSUM") as psF,
        ):
            # ---- PE warm-up: transposes on a memset scratch tile, no DMA
            # deps, so they run the moment the preamble ends and pull the
            # HAM clock gate to 8/8 before real matmuls arrive. A [1,1]
            # copy of the last result feeds the dbg output (keeps DCE off);
            # its DMA is issued at the very end of the program. ----
            wu_sb = small.tile([P, P], f16, tag="wu_src")
            nc.gpsimd.memset(wu_sb, 0.0)
            wu_ps = None

            def warmup(n):
                nonlocal wu_ps
                for i in range(n):
                    wu_ps = psF.tile([P, P], f32, tag="ps")
                    nc.tensor.matmul(wu_ps, lhsT=wu_sb, rhs=wu_sb, start=True, stop=True)

            warmup(N_WARMUP)
            dbg_sb = small.tile([1, 1], f32, tag="dbg")
            nc.vector.tensor_copy(dbg_sb, wu_ps[0:1, 0:1])

            cc_sb = consts.tile([P, CC_COLS], f32)
            nc.sync.dma_start(cc_sb, cpack)
            iota_sb = cc_sb[:, 0:64].bitcast(f16)
            w3_sb = cc_sb[:, 64:67].bitcast(f16)
            b1_sb = cc_sb[:, 67:73]
            b2_sb = cc_sb[:, 73:79]
            b3_sb = cc_sb[0:1, 79:80]
            b3c_sb = cc_sb[:, 79:80]
            sid_sb = cc_sb[:, 80:112]
            ivt_sb = cc_sb[:, 112:144]

            # ---- hidden + weight streaming on sync/HWDGE, ordered to match
            # the compute pipeline ----
            hbs = [None] * BL
            w1ks = [None] * KH
            w2ks = [None] * KH

            def load_hb(b):
                hb = hpool.tile(
                    [P, KT, H], f16, tag=f"hb{b % 5}", name=f"hb{b}"
                )
                nc.sync.dma_start(hb, hid[b].rearrange("(p k) h -> p k h", p=P))
                hbs[b] = hb

            def load_w(ws, wdram, k0, nm):
                # one DMA for three k-tiles
                t = wpool.tile([P, 3, H], f16, tag=f"{nm}{k0}", name=f"{nm}{k0}")
                nc.sync.dma_start(
                    t, wdram[k0 * P : (k0 + 3) * P, :].rearrange("(k p) h -> p k h", p=P)
                )
                for k in range(3):
                    ws[k0 + k] = t[:, k, :]

            load_hb(0)
            load_hb(1)
            load_w(w1ks, w1, 0, "w1k")
            load_w(w1ks, w1, 3, "w1k")
            load_hb(2)
            load_hb(3)
            load_hb(4)
            load_w(w2ks, w2, 0, "w2k")
            load_hb(5)
            load_w(w2ks, w2, 3, "w2k")
            load_hb(6)
            load_hb(7)

            xts = [xtpool.tile([P, R], f16, tag=f"xt{k}", name=f"xt{k}") for k in range(KH)]
            y1s = [ypool.tile([P, R], f16, tag=f"y1_{m}", name=f"y1_{m}") for m in range(KH)]
            y2s = [ypool.tile([P, R], f16, tag=f"y2_{m}", name=f"y2_{m}") for m in range(KH)]
            pred4 = ypool.tile([P, R // 4], f32, tag="pred")

            def pool(b):
                # scaled one-hots for this batch's four t-tiles
                mts = []
                for k in range(KT):
                    mt = mtpool.tile([P, P], f16, tag="mt")
                    nc.vector.tensor_scalar(
                        mt, iota_sb,
                        sid_sb[:, b * KT + k : b * KT + k + 1],
                        ivt_sb[:, b * KT + k : b * KT + k + 1],
                        OP.is_equal, OP.mult,
                    )
                    mts.append(mt)
                # pooled^T m-blocks: psum banks rotate m%3 so the per-m copy
                # (on the other engines) never touches the bank PE is writing
                pts = [
                    psA.tile([P, 2 * P], f32, tag="psA", name=f"pool{b}_{j}")
                    for j in range(3)
                ]
                for m in range(KH):
                    dst = pts[m % 3][:, (m // 3) * P : (m // 3 + 1) * P]
                    for k in range(KT):
                        nc.tensor.matmul(
                            dst,
                            lhsT=hbs[b][:, k, m * P : (m + 1) * P],
                            rhs=mts[k],
                            start=(k == 0),
                            stop=(k == KT - 1),
                        )
                    # copy this m-block out while the next m-blocks stream
                    if m % 2 == 0:
                        nc.vector.tensor_copy(xts[m][:, b * S : (b + 1) * S], dst)
                    else:
                        nc.scalar.copy(xts[m][:, b * S : (b + 1) * S], dst)

            def fc(wks, b_sb, xs, outs, rc, RC, func):
                for m in range(KH):
                    pt = psF.tile([P, RC], f32, tag="ps")
                    for k in range(KH):
                        nc.tensor.matmul(
                            pt,
                            lhsT=wks[k][:, m * P : (m + 1) * P],
                            rhs=xs[k][:, rc * RC : (rc + 1) * RC],
                            start=(k == 0),
                            stop=(k == KH - 1),
                        )
                    nc.scalar.activation(
                        outs[m][:, rc * RC : (rc + 1) * RC],
                        pt,
                        func,
                        bias=b_sb[:, m : m + 1],
                    )

            def fc3_all():
                # 4 independent col-group accumulations run concurrently on
                # the four 32-wide array strips; each group's output row sits
                # at psum partition 32j of the same bank (groups sequential
                # per slice, so has_written semantics stay clean)
                RQ = R // 4
                pt4 = psF.tile([P, RQ], f32, tag="ps")
                for j in range(4):
                    for k in range(KH):
                        nc.tensor.matmul(
                            pt4[32 * j : 32 * j + 1, :],
                            lhsT=w3_sb[:, k : k + 1],
                            rhs=y2s[k][:, j * RQ : (j + 1) * RQ],
                            start=(k == 0),
                            stop=(k == KH - 1),
                            tile_position=(0, 32 * j),
                        )
                # ACT rejects partition-strided APs: one sigmoid per strip
                # row (they pipeline); the DMA reads all four rows at once
                for j in range(4):
                    nc.scalar.activation(
                        pred4[32 * j : 32 * j + 1, :],
                        pt4[32 * j : 32 * j + 1, :],
                        FT.Sigmoid,
                        bias=b3c_sb[32 * j : 32 * j + 1, :],
                    )
                strip = lambda ap: ap.rearrange("(j r) c -> j r c", j=4)[:, 0, :]
                nc.sync.dma_start(
                    out.rearrange("b s -> (b s)").rearrange("(j c) -> j c", j=4),
                    strip(pred4),
                )

            gelu = FT.Gelu
            pool(0)
            pool(1)
            fc(w1ks, b1_sb, xts, y1s, 0, RC1, gelu)
            pool(2)
            pool(3)
            fc(w1ks, b1_sb, xts, y1s, 1, RC1, gelu)
            pool(4)
            pool(5)
            fc(w1ks, b1_sb, xts, y1s, 2, RC1, gelu)
            pool(6)
            fc(w2ks, b2_sb, y1s, y2s, 0, RC2, gelu)
            pool(7)
            fc(w1ks, b1_sb, xts, y1s, 3, RC1, gelu)
            fc(w2ks, b2_sb, y1s, y2s, 1, RC2, gelu)
            fc3_all()

            nc.sync.dma_start(dbg, dbg_sb)

    nc.compile()
    return nc


def _get_program():
    if "nc" not in _CACHE:
        _CACHE["nc"] = _build_program()
    return _CACHE["nc"]


def _cpack(sid_shard, b1, b2, b3, w3):
    """Merged per-core constant pack (one DMA): fp16 matmul operands and
    f32 biases/sid/invtok, byte-concatenated per partition row."""
    h16 = np.zeros((P, P + KH), dtype=np.float16)
    h16[:, 0:P] = np.arange(P, dtype=np.float16)[None, :]
    h16[:, P:] = (
        np.asarray(w3, np.float32).reshape(KH, P, 1)[:, :, 0].T.astype(np.float16)
    )
    f = np.zeros((P, 77), dtype=np.float32)
    f[:, 0:6] = np.asarray(b1, np.float32).reshape(KH, P).T
    f[:, 6:12] = np.asarray(b2, np.float32).reshape(KH, P).T
    f[[0, 32, 64, 96], 12] = np.float32(np.asarray(b3).reshape(-1)[0])
    # token layout matches the hidden DMA: partition p, col k = token 4p+k
    for b in range(BL):
        sid_pk = sid_shard[b].astype(np.int64).reshape(P, KT)
        f[:, 13 + b * KT : 13 + (b + 1) * KT] = sid_pk.astype(np.float32)
        # per-token inverse segment size (counts are metadata of the int32
        # index input): invtok[t] = 1/count[sid[t]]
        cnt = np.bincount(sid_shard[b].astype(np.int64), minlength=S)[:S]
        invb = 1.0 / np.maximum(cnt, 1).astype(np.float32)
        f[:, 45 + b * KT : 45 + (b + 1) * KT] = invb[sid_pk]
    row_bytes = np.concatenate(
        [h16.view(np.uint8).reshape(P, -1), f.view(np.uint8).reshape(P, -1)], axis=1
    )
    return np.ascontiguousarray(row_bytes).view(np.float32)


def make_in_maps(hidden, statements_ids, w1, b1, w2, b2, w3, b3):
    hidden = np.ascontiguousarray(
        np.asarray(hidden, dtype=np.float32).astype(np.float16)
    )
    sid = np.asarray(statements_ids, dtype=np.int32)
    w1 = np.ascontiguousarray(np.asarray(w1, dtype=np.float32).astype(np.float16))
    w2 = np.ascontiguousarray(np.asarray(w2, dtype=np.float32).astype(np.float16))
    in_maps = []
    for c in range(N_CORES):
        cc = _cpack(sid[c * BL : (c + 1) * BL], b1, b2, b3, w3)
        in_maps.append(
            {
                "hidden": hidden[c * BL : (c + 1) * BL],
                "w1": w1,
                "w2": w2,
                "cpack": cc,
            }
        )
    return in_maps


def kernel(hidden, statements_ids, w1, b1, w2, b2, w3, b3, **kwargs):
    nc = _get_program()
    in_maps = make_in_maps(hidden, statements_ids, w1, b1, w2, b2, w3, b3)
    trace = bool(int(os.environ.get("KERNEL_TRACE", "0")))
    res = bass_utils.run_bass_kernel_spmd(
        nc, in_maps, core_ids=list(range(N_CORES)), trace=trace
    )
    _CACHE["last_results"] = res
    out = np.concatenate([res.results[c]["out"] for c in range(N_CORES)], axis=0)
    return out.astype(np.float32)
